# revision 21
# baseline (speedup 1.0000x reference)
"""Trainium2 Bass kernel for nn_Attention (dense transformer block-attention).

Reference semantics (faithful reshape WITHOUT head transpose):
  qkv = x @ w_qkv                    # [B, N, 3*1024]
  head h <- token rows [h*128,(h+1)*128); pseudo-token n = r*16 + cb,
  q_head[n, dd] = q[h*128 + r, cb*64 + dd]   (cb in [0,16), dd in [0,64))

Sharding: 32 (b, head) pairs over 8 cores -> each core: 1 batch x 4 heads.
Pure data parallel, no collectives. Host preps xT (bf16) per core + full w
(bf16), and performs the final softmax normalization (divide by the
denominator row that rides along the PV accumulation).

Key structure (v3):
- q/k projection computed TRANSPOSED (w stationary, xT moving): qT/kT come
  out of PSUM directly in [d, token] layout, no PE transposes.  A 128-col
  w c-tile covers col-blocks (2m, 2m+1): psum parts 0-63 hold d of the
  even cb, parts 64-127 d of the odd cb.  kTs = partition-half-swapped
  copy of kT (SBUF->SBUF DMA).
- S matmuls contract K=64 as row-tiled concurrent pairs; both tiles use
  the SAME key block j (lo half from kT/kTs, hi half from the other), so
  the step's two PV matmuls share one v stationary (fewer LDWEIGHTS
  serializations).  Tile (0,0) covers even-cb queries, (64,0) odd-cb.
- exp on ACT in [128,1024] chunks (ping-pong 2x2 PSUM banks), bf16 out.
- PV: out^T = [v|ones].T @ exp(S^T): softmax denominators ride in row 64.
  Unnormalized [65, 512] bf16 tiles are DMA'd out; host divides.
- ~40 warm-up matmuls on dummy data run from t=0 so the PE HAM clock
  gate reaches 2.4 GHz before the real projection starts.
- Projection is split into head-pair halves (N=256) emitted as 4-matmul
  granules paced one per attention step, just-in-time for first use.
"""

import numpy as np
import ml_dtypes

B, N, D = 2, 2048, 1024
H_PER_CORE = 4
ROWS = 128
DH = 64
CB = 16
SCALE = 0.125            # 64 ** -0.5
N_CORES = 8
KO = D // 128            # 8 k-tiles

_GRAPH = None


def build_graph():
    global _GRAPH
    if _GRAPH is not None:
        return _GRAPH

    import concourse.mybir as mybir
    import concourse.tile as tile
    from concourse import bacc
    from contextlib import ExitStack

    f32 = mybir.dt.float32
    bf16 = mybir.dt.bfloat16
    EXP = mybir.ActivationFunctionType.Exp

    nc = bacc.Bacc("TRN2", target_bir_lowering=False, debug=False,
                   num_devices=N_CORES)

    xt_dram = nc.dram_tensor("xt", [D, H_PER_CORE * ROWS], bf16,
                             kind="ExternalInput")
    w_dram = nc.dram_tensor("w", [D, 3 * D], bf16, kind="ExternalInput")
    out_dram = nc.dram_tensor("out", [DH + 1, 8192], bf16,
                              kind="ExternalOutput")

    with tile.TileContext(nc) as tc, ExitStack() as ctx:
        const_pool = ctx.enter_context(tc.tile_pool(name="const", bufs=1))
        in_pool = ctx.enter_context(tc.tile_pool(name="inputs", bufs=1))
        head_pool = ctx.enter_context(tc.tile_pool(name="head", bufs=1))
        pt_pool = ctx.enter_context(tc.tile_pool(name="pt", bufs=10))
        ot_pool = ctx.enter_context(tc.tile_pool(name="ot", bufs=4))
        psA = ctx.enter_context(tc.tile_pool(name="psA", bufs=2,
                                             space="PSUM"))
        psPO = ctx.enter_context(tc.tile_pool(name="psPO", bufs=1,
                                              space="PSUM"))
        psPJ = ctx.enter_context(tc.tile_pool(name="psPJ", bufs=2,
                                              space="PSUM"))

        # warm the exp table immediately
        warm = const_pool.tile([128, 1], f32, tag="warm")
        nc.vector.memset(warm[:], 0.0)
        nc.scalar.activation(warm[:], warm[:], EXP)

        # ---- PE warm-up: dummy matmuls keep the HAM clock gate busy while
        # the input DMA streams in, so real work starts at 2.4 GHz.
        wu = const_pool.tile([128, 512], bf16, tag="wu")
        nc.vector.memset(wu[:], 0.0)
        for i in range(12):
            pw = psPJ.tile([128, 512], f32, tag="pj")
            nc.tensor.matmul(pw[:], wu[:, 0:128], wu[:], start=True,
                             stop=True)

        # ---- input DMA in first-consumption order ----
        xt_sbuf = in_pool.tile([128, KO, H_PER_CORE * ROWS], bf16, tag="xt")
        w_sbuf = in_pool.tile([128, KO, 3 * D], bf16, tag="w")
        # Input DMA, issued striped across four engine sequencers: a
        # single sequencer spends ~650ns issuing each dma_start, so serial
        # issue of ~56 chunks would gate the whole startup (~36us).
        # Critical-path chunks first (small = low latency), bulk after.
        # Sized so contiguous runs stay >= 256B-1.8KB; striped across the
        # three DMA-capable engine sequencers for parallel issue.
        def wchunk(ko, c0, c1):
            return (w_sbuf[:, ko, c0:c1],
                    w_dram.ap()[ko * 128:(ko + 1) * 128, c0:c1])

        chunks_a = []
        for ko in range(KO):
            chunks_a.append(wchunk(ko, 1024, 1536))          # k c-tiles 0-3
            chunks_a.append((xt_sbuf[:, ko, 0:256],
                             xt_dram.ap()[ko * 128:(ko + 1) * 128, 0:256]))
            chunks_a.append(wchunk(ko, 0, 512))              # q c-tiles 0-3
        for i, (dst, srcap) in enumerate(chunks_a):
            [nc.sync, nc.gpsimd, nc.scalar][i % 3].dma_start(dst, srcap)
        chunks_b = []
        for ko in range(KO):
            chunks_b.append(wchunk(ko, 2048, 2560))          # v half 0
        for ko in range(KO):
            chunks_b.append((xt_sbuf[:, ko, 256:512],
                             xt_dram.ap()[ko * 128:(ko + 1) * 128, 256:512]))
        for ko in range(KO):
            chunks_b.append(wchunk(ko, 1536, 2048))          # k c-tiles 4-7
        for ko in range(KO):
            chunks_b.append(wchunk(ko, 2560, 3072))          # v half 1
        for ko in range(KO):
            chunks_b.append(wchunk(ko, 512, 1024))           # q c-tiles 4-7
        for i, (dst, srcap) in enumerate(chunks_b):
            [nc.sync, nc.gpsimd][i % 2].dma_start(dst, srcap)

        # persistent tiles: [part, m, t, r]; parts 0-63 = d of cb 2m,
        # parts 64-127 = d of cb 2m+1 (kT/qT); kTs = halves swapped.
        qT = head_pool.tile([128, 8, 4, 128], bf16, tag="qT", name="qT")
        kT = head_pool.tile([128, 8, 4, 128], bf16, tag="kT", name="kT")
        kTs = head_pool.tile([128, 8, 4, 128], bf16, tag="kTs", name="kTs")
        v_ones = [head_pool.tile([128, CB, DH + 1], bf16, tag=f"vo{t}",
                                 name=f"vo{t}")
                  for t in range(H_PER_CORE)]
        for t in range(H_PER_CORE):
            nc.vector.memset(v_ones[t][:, :, DH], 1.0)

        # ---- projection, emitted in 4-matmul granules ----
        pj_state = {}

        def proj_qk_granule(sec, m, hp, g):
            # sec: 0 = q, 1 = k; m: c-tile; hp: head pair (2 = all four
            # heads at once); g: ko half
            key = (sec, m, hp)
            full = hp == 2
            r0, r1, t0, t1 = (0, 512, 0, 4) if full else \
                (hp * 256, (hp + 1) * 256, hp * 2, (hp + 1) * 2)
            if g == 0:
                pj_state[key] = psPJ.tile([128, r1 - r0], f32, tag="pj",
                                          name=f"pj_{sec}_{m}_{hp}")
            ps = pj_state[key]
            c0 = sec * 1024 + m * 128
            for ko in range(g * 4, g * 4 + 4):
                nc.tensor.matmul(
                    ps[:],
                    w_sbuf[:, ko, c0:c0 + 128],
                    xt_sbuf[:, ko, r0:r1],
                    start=(ko == 0), stop=(ko == KO - 1))
            if g == 1:
                dst = qT if sec == 0 else kT
                nc.vector.tensor_copy(dst[:, m, t0:t1, :], ps[:])
                del pj_state[key]
                if sec == 1:
                    # partition-half-swapped copy, straight from PSUM on DVE
                    nc.vector.tensor_copy(
                        kTs[64:128, m, t0:t1, :], ps[0:64, :])
                    nc.vector.tensor_copy(
                        kTs[0:64, m, t0:t1, :], ps[64:128, :])

        def proj_v_granule(t, half, g):
            key = ("v", t, half)
            if g == 0:
                pj_state[key] = psPJ.tile([128, 512], f32, tag="pj",
                                          name=f"pjv_{t}_{half}")
            ps = pj_state[key]
            c0 = 2048 + half * 512
            for ko in range(g * 4, g * 4 + 4):
                nc.tensor.matmul(
                    ps[:],
                    xt_sbuf[:, ko, t * 128:(t + 1) * 128],
                    w_sbuf[:, ko, c0:c0 + 512],
                    start=(ko == 0), stop=(ko == KO - 1))
            if g == 1:
                src = ps[:].rearrange("p (a b) -> p a b", b=DH)
                nc.vector.tensor_copy(
                    v_ones[t][:, half * 8:(half + 1) * 8, 0:DH], src)
                del pj_state[key]

        def proj_qk(sec, m, hp):
            proj_qk_granule(sec, m, hp, 0)
            proj_qk_granule(sec, m, hp, 1)

        # filler granules, in need-order; paced via PLAN below
        fillers = []
        for mm in range(1, 8):
            fillers += [("k", mm, 0, 0), ("k", mm, 0, 1)]       # 14
        for mm in range(4, 8):
            fillers += [("q", mm, 0, 0), ("q", mm, 0, 1)]       # 8
        fillers += [("v", 1, 0, 0), ("v", 1, 0, 1),
                    ("v", 1, 1, 0), ("v", 1, 1, 1)]             # 4
        for mm in range(1, 8):
            fillers += [("k", mm, 1, 0), ("k", mm, 1, 1)]       # 14
        for mm in range(4, 8):
            fillers += [("q", mm, 1, 0), ("q", mm, 1, 1)]       # 8
        fillers += [("v", 2, 0, 0), ("v", 2, 0, 1),
                    ("v", 2, 1, 0), ("v", 2, 1, 1)]             # 4
        fillers += [("v", 3, 0, 0), ("v", 3, 0, 1),
                    ("v", 3, 1, 0), ("v", 3, 1, 1)]             # 4
        fillers.reverse()

        def emit_filler():
            if fillers:
                kind, a, b, g = fillers.pop()
                if kind == "v":
                    proj_v_granule(a, b, g)
                else:
                    proj_qk_granule(0 if kind == "q" else 1, a, b, g)

        # granules allowed per (t, s, j) step.  (0,s0): k h01 just-in-time
        # early, then 2/step so q h01 ct4-7 land before s=1 needs them.
        def allowance(t, s, j):
            if t == 0 and s == 0:
                return 1 if j < 10 else 2
            if t in (0, 1):
                return 1
            if t == 2 and s == 0:
                return 1
            return 0

        # ---- attention step ----
        def step(t, m, odd, s, po_e, po_o, first, last):
            lo_st = (kTs if odd else kT)[0:64, m, t, :]
            hi_st = (kT if odd else kTs)[64:128, m, t, :]
            ps = psA.tile([128, 1024], f32, tag="psA")
            nc.tensor.matmul(ps[:, 0:512], lo_st,
                             qT[0:64, s * 4:(s + 1) * 4, t, :],
                             start=True, stop=True)
            nc.tensor.matmul(ps[:, 512:1024], hi_st,
                             qT[64:128, s * 4:(s + 1) * 4, t, :],
                             start=True, stop=True)
            pt = pt_pool.tile([128, 1024], bf16, tag="pt")
            nc.scalar.activation(pt[:], ps[:], EXP, scale=SCALE)
            if t == 0 and s == 0 and odd == 0 and m in (0, 4):
                # v0 projection halves, each emitted just before the first
                # PV that needs them (RAW dep in program order); the
                # scheduler floats them to when the v-cols arrive while
                # later S/exp steps keep flowing.
                half = 0 if m == 0 else 1
                proj_v_granule(0, half, 0)
                proj_v_granule(0, half, 1)
            j = 2 * m + odd
            nc.tensor.matmul(po_e[:], v_ones[t][:, j, :],
                             pt[:, 0:512], start=first, stop=last)
            nc.tensor.matmul(po_o[:], v_ones[t][:, j, :],
                             pt[:, 512:1024], start=first, stop=last)

        def evac(t, s, po_e, po_o):
            last = (t == H_PER_CORE - 1 and s == 1)
            for p, po in ((0, po_e), (1, po_o)):
                ot = ot_pool.tile([DH + 1, 512], bf16, tag="ot")
                if last and p == 1:
                    nc.scalar.copy(ot[:], po[:])
                else:
                    nc.vector.tensor_copy(ot[:], po[:])
                base = ((t * 2 + p) * 2 + s) * 512
                nc.sync.dma_start(out_dram.ap()[0:DH + 1, base:base + 256],
                                  ot[:, 0:256])
                nc.sync.dma_start(
                    out_dram.ap()[0:DH + 1, base + 256:base + 512],
                    ot[:, 256:512])

        # ---- startup projection (minimum prerequisites for head 0) ----
        proj_qk(1, 0, 2)                 # kT c-tile 0, all heads (+ swap)
        for mm in range(4):
            proj_qk(0, mm, 2)            # qT c-tiles 0-3, all heads


        # ---- main attention loop ----
        for t in range(H_PER_CORE):
            for s in range(2):
                po_e = psPO.tile([DH + 1, 512], f32, tag="poe")
                po_o = psPO.tile([DH + 1, 512], f32, tag="poo")
                for j in range(16):
                    step(t, j // 2, j % 2, s, po_e, po_o,
                         first=(j == 0), last=(j == 15))
                    for _ in range(allowance(t, s, j)):
                        emit_filler()
                evac(t, s, po_e, po_o)

    nc.compile()
    _GRAPH = nc
    return nc


def make_in_maps(x, w_qkv):
    w_bf = np.ascontiguousarray(w_qkv).astype(ml_dtypes.bfloat16)
    maps = []
    for c in range(N_CORES):
        b = c // 4
        r0 = (c % 4) * H_PER_CORE * ROWS
        xt = np.ascontiguousarray(
            x[b, r0:r0 + H_PER_CORE * ROWS, :].T).astype(ml_dtypes.bfloat16)
        maps.append({"xt": xt, "w": w_bf})
    return maps


def assemble_out(results):
    out = np.empty((B, N, D), dtype=np.float32)
    for c in range(N_CORES):
        b, quad = divmod(c, 4)
        arr = np.asarray(results[c]["out"]).astype(np.float32)
        arr = arr.reshape(DH + 1, 4, 2, 2, 4, 128)   # [part,t,p,s,mi,r]
        num = arr[0:DH]
        den = arr[DH]
        ratio = num / den[None]                      # [dd,t,p,s,mi,r]
        tmp = ratio.transpose(1, 5, 3, 4, 2, 0)      # [t,r,s,mi,p,dd]
        out[b, quad * 512:(quad + 1) * 512, :] = tmp.reshape(512, 1024)
    return out


def kernel(x, w_qkv):
    from concourse import bass_utils
    nc = build_graph()
    res = bass_utils.run_bass_kernel_spmd(
        nc, make_in_maps(np.asarray(x), np.asarray(w_qkv)),
        list(range(N_CORES)))
    return assemble_out(res.results)


# revision 22
# speedup vs baseline: 1.0438x; 1.0438x over previous
"""Trainium2 Bass kernel for nn_Attention (dense transformer block-attention).

Reference semantics (faithful reshape WITHOUT head transpose):
  qkv = x @ w_qkv                    # [B, N, 3*1024]
  head h <- token rows [h*128,(h+1)*128); pseudo-token n = r*16 + cb,
  q_head[n, dd] = q[h*128 + r, cb*64 + dd]   (cb in [0,16), dd in [0,64))

Sharding: 32 (b, head) pairs over 8 cores -> each core: 1 batch x 4 heads.
Pure data parallel, no collectives. Host preps xT (bf16) per core + full w
(bf16), and performs the final softmax normalization (divide by the
denominator row that rides along the PV accumulation).

Key structure (v3):
- q/k projection computed TRANSPOSED (w stationary, xT moving): qT/kT come
  out of PSUM directly in [d, token] layout, no PE transposes.  A 128-col
  w c-tile covers col-blocks (2m, 2m+1): psum parts 0-63 hold d of the
  even cb, parts 64-127 d of the odd cb.  kTs = partition-half-swapped
  copy of kT (SBUF->SBUF DMA).
- S matmuls contract K=64 as row-tiled concurrent pairs; both tiles use
  the SAME key block j (lo half from kT/kTs, hi half from the other), so
  the step's two PV matmuls share one v stationary (fewer LDWEIGHTS
  serializations).  Tile (0,0) covers even-cb queries, (64,0) odd-cb.
- exp on ACT in [128,1024] chunks (ping-pong 2x2 PSUM banks), bf16 out.
- PV: out^T = [v|ones].T @ exp(S^T): softmax denominators ride in row 64.
  Unnormalized [65, 512] bf16 tiles are DMA'd out; host divides.
- ~40 warm-up matmuls on dummy data run from t=0 so the PE HAM clock
  gate reaches 2.4 GHz before the real projection starts.
- Projection is split into head-pair halves (N=256) emitted as 4-matmul
  granules paced one per attention step, just-in-time for first use.
"""

import numpy as np
import ml_dtypes

B, N, D = 2, 2048, 1024
H_PER_CORE = 4
ROWS = 128
DH = 64
CB = 16
SCALE = 0.125            # 64 ** -0.5
N_CORES = 8
KO = D // 128            # 8 k-tiles

_GRAPH = None


def build_graph():
    global _GRAPH
    if _GRAPH is not None:
        return _GRAPH

    import concourse.mybir as mybir
    import concourse.tile as tile
    from concourse import bacc
    from contextlib import ExitStack

    f32 = mybir.dt.float32
    bf16 = mybir.dt.bfloat16
    EXP = mybir.ActivationFunctionType.Exp

    nc = bacc.Bacc("TRN2", target_bir_lowering=False, debug=False,
                   num_devices=N_CORES)

    xt_dram = nc.dram_tensor("xt", [D, H_PER_CORE * ROWS], bf16,
                             kind="ExternalInput")
    w_dram = nc.dram_tensor("w", [D, 3 * D], bf16, kind="ExternalInput")
    out_dram = nc.dram_tensor("out", [DH + 1, 8192], bf16,
                              kind="ExternalOutput")

    with tile.TileContext(nc) as tc, ExitStack() as ctx:
        const_pool = ctx.enter_context(tc.tile_pool(name="const", bufs=1))
        in_pool = ctx.enter_context(tc.tile_pool(name="inputs", bufs=1))
        head_pool = ctx.enter_context(tc.tile_pool(name="head", bufs=1))
        pt_pool = ctx.enter_context(tc.tile_pool(name="pt", bufs=10))
        ot_pool = ctx.enter_context(tc.tile_pool(name="ot", bufs=4))
        psA = ctx.enter_context(tc.tile_pool(name="psA", bufs=2,
                                             space="PSUM"))
        psPO = ctx.enter_context(tc.tile_pool(name="psPO", bufs=1,
                                              space="PSUM"))
        psPJ = ctx.enter_context(tc.tile_pool(name="psPJ", bufs=2,
                                              space="PSUM"))

        # warm the exp table immediately
        warm = const_pool.tile([128, 1], f32, tag="warm")
        nc.vector.memset(warm[:], 0.0)
        nc.scalar.activation(warm[:], warm[:], EXP)

        # ---- PE warm-up: dummy matmuls keep the HAM clock gate busy while
        # the input DMA streams in, so real work starts at 2.4 GHz.
        wu = const_pool.tile([128, 512], bf16, tag="wu")
        nc.vector.memset(wu[:], 0.0)
        for i in range(12):
            pw = psPJ.tile([128, 512], f32, tag="pj")
            nc.tensor.matmul(pw[:], wu[:, 0:128], wu[:], start=True,
                             stop=True)

        # ---- input DMA in first-consumption order ----
        xt_sbuf = in_pool.tile([128, KO, H_PER_CORE * ROWS], bf16, tag="xt")
        w_sbuf = in_pool.tile([128, KO, 3 * D], bf16, tag="w")
        # Input DMA, issued striped across four engine sequencers: a
        # single sequencer spends ~650ns issuing each dma_start, so serial
        # issue of ~56 chunks would gate the whole startup (~36us).
        # Critical-path chunks first (small = low latency), bulk after.
        # Sized so contiguous runs stay >= 256B-1.8KB; striped across the
        # three DMA-capable engine sequencers for parallel issue.
        def wchunk(ko, c0, c1):
            return (w_sbuf[:, ko, c0:c1],
                    w_dram.ap()[ko * 128:(ko + 1) * 128, c0:c1])

        chunks_a = []
        for ko in range(KO):
            chunks_a.append(wchunk(ko, 1024, 1536))          # k c-tiles 0-3
            chunks_a.append((xt_sbuf[:, ko, 0:256],
                             xt_dram.ap()[ko * 128:(ko + 1) * 128, 0:256]))
            chunks_a.append(wchunk(ko, 0, 512))              # q c-tiles 0-3
        for i, (dst, srcap) in enumerate(chunks_a):
            [nc.sync, nc.gpsimd, nc.scalar][i % 3].dma_start(dst, srcap)
        chunks_b = []
        for ko in range(KO):
            chunks_b.append(wchunk(ko, 2048, 2560))          # v half 0
        for ko in range(KO):
            chunks_b.append((xt_sbuf[:, ko, 256:512],
                             xt_dram.ap()[ko * 128:(ko + 1) * 128, 256:512]))
        for ko in range(KO):
            chunks_b.append(wchunk(ko, 1536, 2048))          # k c-tiles 4-7
        for ko in range(KO):
            chunks_b.append(wchunk(ko, 2560, 3072))          # v half 1
        for ko in range(KO):
            chunks_b.append(wchunk(ko, 512, 1024))           # q c-tiles 4-7
        for i, (dst, srcap) in enumerate(chunks_b):
            [nc.sync, nc.gpsimd][i % 2].dma_start(dst, srcap)

        # persistent tiles: [part, m, t, r]; parts 0-63 = d of cb 2m,
        # parts 64-127 = d of cb 2m+1 (kT/qT); kTs = halves swapped.
        qT = head_pool.tile([128, 8, 4, 128], bf16, tag="qT", name="qT")
        kT = head_pool.tile([128, 8, 4, 128], bf16, tag="kT", name="kT")
        kTs = head_pool.tile([128, 8, 4, 128], bf16, tag="kTs", name="kTs")
        v_ones = [head_pool.tile([128, CB, DH + 1], bf16, tag=f"vo{t}",
                                 name=f"vo{t}")
                  for t in range(H_PER_CORE)]
        for t in range(H_PER_CORE):
            nc.vector.memset(v_ones[t][:, :, DH], 1.0)

        # ---- projection, emitted in 4-matmul granules ----
        pj_state = {}

        def proj_qk_granule(sec, m, hp, g):
            # sec: 0 = q, 1 = k; m: c-tile; hp: head pair; g: ko half
            key = (sec, m, hp)
            if g == 0:
                pj_state[key] = psPJ.tile([128, 256], f32, tag="pj",
                                          name=f"pj_{sec}_{m}_{hp}")
            ps = pj_state[key]
            c0 = sec * 1024 + m * 128
            for ko in range(g * 4, g * 4 + 4):
                nc.tensor.matmul(
                    ps[:],
                    w_sbuf[:, ko, c0:c0 + 128],
                    xt_sbuf[:, ko, hp * 256:(hp + 1) * 256],
                    start=(ko == 0), stop=(ko == KO - 1))
            if g == 1:
                dst = qT if sec == 0 else kT
                nc.vector.tensor_copy(dst[:, m, hp * 2:(hp + 1) * 2, :],
                                      ps[:])
                del pj_state[key]
                if sec == 1:
                    # partition-half-swapped copy, straight from PSUM on DVE
                    nc.vector.tensor_copy(
                        kTs[64:128, m, hp * 2:(hp + 1) * 2, :], ps[0:64, :])
                    nc.vector.tensor_copy(
                        kTs[0:64, m, hp * 2:(hp + 1) * 2, :], ps[64:128, :])

        def proj_v_granule(t, half, g):
            key = ("v", t, half)
            if g == 0:
                pj_state[key] = psPJ.tile([128, 512], f32, tag="pj",
                                          name=f"pjv_{t}_{half}")
            ps = pj_state[key]
            c0 = 2048 + half * 512
            for ko in range(g * 4, g * 4 + 4):
                nc.tensor.matmul(
                    ps[:],
                    xt_sbuf[:, ko, t * 128:(t + 1) * 128],
                    w_sbuf[:, ko, c0:c0 + 512],
                    start=(ko == 0), stop=(ko == KO - 1))
            if g == 1:
                src = ps[:].rearrange("p (a b) -> p a b", b=DH)
                nc.vector.tensor_copy(
                    v_ones[t][:, half * 8:(half + 1) * 8, 0:DH], src)
                del pj_state[key]

        def proj_qk(sec, m, hp):
            proj_qk_granule(sec, m, hp, 0)
            proj_qk_granule(sec, m, hp, 1)

        # filler granules, in need-order; paced via PLAN below
        fillers = []
        for mm in range(1, 8):
            fillers += [("k", mm, 0, 0), ("k", mm, 0, 1)]       # 14
        for mm in range(4, 8):
            fillers += [("q", mm, 0, 0), ("q", mm, 0, 1)]       # 8
        fillers += [("v", 1, 0, 0), ("v", 1, 0, 1),
                    ("v", 1, 1, 0), ("v", 1, 1, 1)]             # 4
        for mm in range(8):
            fillers += [("k", mm, 1, 0), ("k", mm, 1, 1)]       # 16
        for mm in range(8):
            fillers += [("q", mm, 1, 0), ("q", mm, 1, 1)]       # 16
        fillers += [("v", 2, 0, 0), ("v", 2, 0, 1),
                    ("v", 2, 1, 0), ("v", 2, 1, 1)]             # 4
        fillers += [("v", 3, 0, 0), ("v", 3, 0, 1),
                    ("v", 3, 1, 0), ("v", 3, 1, 1)]             # 4
        fillers.reverse()

        def emit_filler():
            if fillers:
                kind, a, b, g = fillers.pop()
                if kind == "v":
                    proj_v_granule(a, b, g)
                else:
                    proj_qk_granule(0 if kind == "q" else 1, a, b, g)

        # granules allowed per (t, s, j) step.  (0,s0): k h01 just-in-time
        # early, then 2/step so q h01 ct4-7 land before s=1 needs them.
        def allowance(t, s, j):
            if t == 0 and s == 0:
                return 1 if j < 10 else 2
            if t in (0, 1):
                return 1
            if t == 2 and s == 0:
                return 1
            return 0

        # ---- attention step ----
        def step(t, m, odd, s, po_e, po_o, first, last):
            lo_st = (kTs if odd else kT)[0:64, m, t, :]
            hi_st = (kT if odd else kTs)[64:128, m, t, :]
            ps = psA.tile([128, 1024], f32, tag="psA")
            nc.tensor.matmul(ps[:, 0:512], lo_st,
                             qT[0:64, s * 4:(s + 1) * 4, t, :],
                             start=True, stop=True)
            nc.tensor.matmul(ps[:, 512:1024], hi_st,
                             qT[64:128, s * 4:(s + 1) * 4, t, :],
                             start=True, stop=True)
            pt = pt_pool.tile([128, 1024], bf16, tag="pt")
            nc.scalar.activation(pt[:], ps[:], EXP, scale=SCALE)
            if t == 0 and s == 0 and odd == 0 and m in (0, 4):
                # v0 projection halves, each emitted just before the first
                # PV that needs them (RAW dep in program order); the
                # scheduler floats them to when the v-cols arrive while
                # later S/exp steps keep flowing.
                half = 0 if m == 0 else 1
                proj_v_granule(0, half, 0)
                proj_v_granule(0, half, 1)
            j = 2 * m + odd
            nc.tensor.matmul(po_e[:], v_ones[t][:, j, :],
                             pt[:, 0:512], start=first, stop=last)
            nc.tensor.matmul(po_o[:], v_ones[t][:, j, :],
                             pt[:, 512:1024], start=first, stop=last)

        def evac(t, s, po_e, po_o):
            last = (t == H_PER_CORE - 1 and s == 1)
            for p, po in ((0, po_e), (1, po_o)):
                ot = ot_pool.tile([DH + 1, 512], bf16, tag="ot")
                if last and p == 1:
                    nc.scalar.copy(ot[:], po[:])
                else:
                    nc.vector.tensor_copy(ot[:], po[:])
                base = ((t * 2 + p) * 2 + s) * 512
                nc.sync.dma_start(out_dram.ap()[0:DH + 1, base:base + 256],
                                  ot[:, 0:256])
                nc.sync.dma_start(
                    out_dram.ap()[0:DH + 1, base + 256:base + 512],
                    ot[:, 256:512])

        # ---- startup projection (minimum prerequisites for head 0) ----
        proj_qk(1, 0, 0)                 # kT c-tile 0, heads 0-1 (+ swap)
        for mm in range(4):
            proj_qk(0, mm, 0)            # qT c-tiles 0-3, heads 0-1


        # ---- main attention loop ----
        for t in range(H_PER_CORE):
            for s in range(2):
                po_e = psPO.tile([DH + 1, 512], f32, tag="poe")
                po_o = psPO.tile([DH + 1, 512], f32, tag="poo")
                for j in range(16):
                    step(t, j // 2, j % 2, s, po_e, po_o,
                         first=(j == 0), last=(j == 15))
                    for _ in range(allowance(t, s, j)):
                        emit_filler()
                evac(t, s, po_e, po_o)

    nc.compile()
    _GRAPH = nc
    return nc


def make_in_maps(x, w_qkv):
    w_bf = np.ascontiguousarray(w_qkv).astype(ml_dtypes.bfloat16)
    maps = []
    for c in range(N_CORES):
        b = c // 4
        r0 = (c % 4) * H_PER_CORE * ROWS
        xt = np.ascontiguousarray(
            x[b, r0:r0 + H_PER_CORE * ROWS, :].T).astype(ml_dtypes.bfloat16)
        maps.append({"xt": xt, "w": w_bf})
    return maps


def assemble_out(results):
    out = np.empty((B, N, D), dtype=np.float32)
    for c in range(N_CORES):
        b, quad = divmod(c, 4)
        arr = np.asarray(results[c]["out"]).astype(np.float32)
        arr = arr.reshape(DH + 1, 4, 2, 2, 4, 128)   # [part,t,p,s,mi,r]
        num = arr[0:DH]
        den = arr[DH]
        ratio = num / den[None]                      # [dd,t,p,s,mi,r]
        tmp = ratio.transpose(1, 5, 3, 4, 2, 0)      # [t,r,s,mi,p,dd]
        out[b, quad * 512:(quad + 1) * 512, :] = tmp.reshape(512, 1024)
    return out


def kernel(x, w_qkv):
    from concourse import bass_utils
    nc = build_graph()
    res = bass_utils.run_bass_kernel_spmd(
        nc, make_in_maps(np.asarray(x), np.asarray(w_qkv)),
        list(range(N_CORES)))
    return assemble_out(res.results)
